# revision 10
# baseline (speedup 1.0000x reference)
"""Trainium2 Bass kernel for nn_AttnAdapter: GQA attention with RoPE,
region-based enhance/suppress score scaling, causal mask, o_proj.

Sharding: tensor-parallel over heads across 8 NeuronCores. Core d holds
q-heads 4d..4d+3 (wq rows), kv-head d (wk/wv rows), and wo columns
512d..512(d+1). Each core computes a full [S, D] partial of the output;
the host sums the 8 partials (the TP all-reduce, done at unshard time).

v3: all-bf16 matmuls (FWL weight loads), weights loaded once, software-
pipelined emission: projection matmuls for seq-tile j+1 are interleaved
into the attention stream for seq-tile j (and the first o_proj tiles
into the last attention tile) so the PE never stalls on the exp stream.
Softmax denom is accumulated on DVE in bf16, replicated across
partitions with a ones-matmul, and inverted with ACT Ln/Exp (the DVE
reciprocal costs 3.4us/tile). Outputs are stored bf16 and summed on
host.
"""

import math

import numpy as np

# ---- problem constants (hardcoded; kernel.py must be self-contained) ----
S = 2048          # sequence length
D = 4096          # model dim
HD = 128          # head dim
NCORES = 8
QH = 4            # q heads per core
SYS_LEN, IMG_LEN = 35, 576
BOUND = SYS_LEN + IMG_LEN          # 611
ENH, SUP = 1.5, 0.5
ROPE_BASE = 10000.0

J = 4             # sq tiles of 512
NSK = 16          # sk tiles of 128
DCH = 32          # D chunks of 128

_CACHE = {}


def _bf16(x):
    import ml_dtypes
    return np.ascontiguousarray(np.asarray(x, dtype=ml_dtypes.bfloat16))


def _host_constants():
    inv_freq = 1.0 / (ROPE_BASE ** (np.arange(0, HD, 2, dtype=np.float32) / HD))
    pos = np.arange(S, dtype=np.float32)
    freqs = pos[:, None] * inv_freq[None, :]              # [S, 64]
    emb = np.concatenate([freqs, freqs], axis=-1)         # [S, 128]
    cosT = _bf16(np.cos(emb).T)                           # [128, S]
    sinT = _bf16(np.sin(emb).T)

    # rotate_half as a matmul: rot = R @ q (in [hd, s] layout).
    # matmul(out, lhsT, rhs) = lhsT.T @ rhs, so feed RT = R.T.
    RT = np.zeros((HD, HD), dtype=np.float32)
    half = HD // 2
    for c in range(half):
        RT[c + half, c] = -1.0      # rot[c] = -q[c+64]
    for c in range(half, HD):
        RT[c - half, c] = 1.0       # rot[c] = q[c-64]
    rmat = _bf16(RT)

    ident = np.eye(HD, dtype=np.float32)

    # Diagonal-tile causal mask [128, 128]: within the first 128-col block
    # of a diagonal tile, col c valid iff c >= p. Same for every delta.
    p = np.arange(128)[:, None]
    c = np.arange(128)[None, :]
    tri = _bf16((c >= p).astype(np.float32))

    # key_scale in partition layout per sk-tile: ksT[p, i] = scale(128*i+p)
    kpos = np.arange(S)
    key_scale = np.where(kpos < SYS_LEN, SUP,
                         np.where(kpos < BOUND, ENH, 1.0)).astype(np.float32)
    ksT = np.ascontiguousarray(key_scale[:5 * 128].reshape(5, 128).T)  # [128, 5]

    ones128 = _bf16(np.ones((HD, HD), dtype=np.float32))
    return dict(cosT=cosT, sinT=sinT, rmat=rmat, ident=ident, tri=tri,
                ksT=ksT, ones128=ones128)


def _interleave(main, fill):
    """Merge two unit lists, spreading `fill` evenly across `main`."""
    units = []
    nf = len(fill)
    nm = max(1, len(main))
    k = 0
    for m, u in enumerate(main):
        units.append(u)
        want = (m + 1) * nf // nm
        while k < want:
            units.append(fill[k])
            k += 1
    units.extend(fill[k:])
    return units


def _build_bass():
    import concourse.bass as bass
    import concourse.mybir as mybir
    from concourse.tile import TileContext
    from contextlib import ExitStack

    f32 = mybir.dt.float32
    bf16 = mybir.dt.bfloat16

    nc = bass.Bass()
    # xTt[j, d, p, f] = x.T[128d+p, 512j+f] -- each (j,d) tile contiguous
    xTt = nc.dram_tensor("xTt", [J, 8, 128, 2048], bf16, kind="ExternalInput")
    # wkv4[q, p, l*256+c]: c 0:128=wkT chunk of d=4q+l, 128:256=wvT chunk
    wkv4 = nc.dram_tensor("wkv4", [8, 128, 1024], bf16, kind="ExternalInput")
    # wq4[q, p, l*512+c]: wqT chunk (4 heads x 128) of d=4q+l
    wq4 = nc.dram_tensor("wq4", [8, 128, 2048], bf16, kind="ExternalInput")
    # woT[n, h, p, f] = woT[128h+p, 512n+f]
    woT = nc.dram_tensor("woT", [8, 128, 2048], bf16, kind="ExternalInput")
    cosT_d = nc.dram_tensor("cosT", [HD, S], bf16, kind="ExternalInput")
    sinT_d = nc.dram_tensor("sinT", [HD, S], bf16, kind="ExternalInput")
    rmat_d = nc.dram_tensor("rmat", [HD, HD], bf16, kind="ExternalInput")
    ident_d = nc.dram_tensor("ident", [HD, HD], f32, kind="ExternalInput")
    tri_d = nc.dram_tensor("tri", [HD, HD], bf16, kind="ExternalInput")
    ksT_d = nc.dram_tensor("ksT", [HD, 5], f32, kind="ExternalInput")
    ones128_d = nc.dram_tensor("ones128", [HD, HD], bf16, kind="ExternalInput")
    # out[n, t, p, f] = out[128t+p, 512n+f], bf16 partial (host sums cores)
    out = nc.dram_tensor("out", [8, 4, 128, 2048], bf16, kind="ExternalOutput")

    EXP = mybir.ActivationFunctionType.Exp
    LN = mybir.ActivationFunctionType.Ln

    with TileContext(nc) as tc, ExitStack() as ctx:
        const = ctx.enter_context(tc.tile_pool(name="const", bufs=1))
        wpool = ctx.enter_context(tc.tile_pool(name="wpool", bufs=1))
        persist = ctx.enter_context(tc.tile_pool(name="persist", bufs=1))

        wkv = [wpool.tile([128, 1024], bf16, name=f"wkv{q}") for q in range(8)]
        wqt = [wpool.tile([128, 2048], bf16, name=f"wqt{q}") for q in range(8)]
        qrot = [[persist.tile([HD, 512], bf16, name=f"qrot{m}_{j}")
                 for j in range(J)] for m in range(QH)]
        krot = [persist.tile([HD, 512], bf16, name=f"krot{j}") for j in range(J)]
        vnat = [persist.tile([HD, 512], bf16, name=f"vnat{j}") for j in range(J)]
        attn = [[persist.tile([HD, 512], bf16, name=f"attn{h}_{j}")
                 for j in range(J)] for h in range(QH)]

        cosT = const.tile([HD, S], bf16)
        sinT = const.tile([HD, S], bf16)
        rmat = const.tile([HD, HD], bf16)
        ident = const.tile([HD, HD], f32)
        tri = const.tile([HD, HD], bf16)
        ksT = const.tile([HD, 5], f32)
        ones128 = const.tile([HD, HD], bf16)

        with tc.tile_pool(name="xp", bufs=2) as xp, \
             tc.tile_pool(name="wop", bufs=1) as wop, \
             tc.tile_pool(name="accp", bufs=1, space="PSUM") as accp, \
             tc.tile_pool(name="tmpp", bufs=1, space="PSUM") as tmpp, \
             tc.tile_pool(name="sp", bufs=3, space="PSUM") as sp, \
             tc.tile_pool(name="avp", bufs=2, space="PSUM") as avp, \
             tc.tile_pool(name="stage", bufs=1) as stage:

            xt = [[None] * DCH for _ in range(J)]

            def rope_unit(acc, dst, j, eng):
                def emit():
                    sq = slice(j * 512, (j + 1) * 512)
                    q_sb = stage.tile([128, 512], bf16, tag="q_sb", bufs=3)
                    # release copy frees the acc bank; alternate queues so it
                    # is not stuck behind the exp stream
                    if eng == 0:
                        nc.scalar.copy(q_sb[:], acc[:])
                    else:
                        nc.vector.tensor_copy(q_sb[:], acc[:])
                    rot_ps = tmpp.tile([128, 512], f32, tag="tmp")
                    nc.tensor.matmul(rot_ps[:], rmat[:], q_sb[:],
                                     start=True, stop=True)
                    t2 = stage.tile([128, 512], f32, tag="t2", bufs=2)
                    nc.vector.tensor_mul(t2[:], rot_ps[:], sinT[:, sq])
                    nc.vector.tensor_mul(dst[:], q_sb[:], cosT[:, sq])
                    nc.vector.tensor_add(dst[:], dst[:], t2[:])
                return emit

            def a_units(j, with_consts=False):
                """Projection+RoPE units for seq-tile j: (dma_units, main)."""
                dma_units = []
                units = []
                accs = {}

                def dma_unit(q):
                    def emit():
                        t = xp.tile([128, 2048], bf16, tag=f"x{q}",
                                    bufs=2, name=f"x{q}")
                        if j == 0 and q == 0:
                            # fine-grained first chunks: first matmul can
                            # start after ~192KB instead of ~768KB
                            for l in range(4):
                                cs = slice(l * 256, (l + 1) * 256)
                                nc.sync.dma_start(wkv[q][:, cs], wkv4[q, :, cs])
                                xs = slice(l * 512, (l + 1) * 512)
                                nc.sync.dma_start(t[:, xs], xTt[j, q, :, xs])
                        else:
                            if j == 0:
                                nc.sync.dma_start(wkv[q][:], wkv4[q])
                            nc.sync.dma_start(t[:], xTt[j, q])
                        for l in range(4):
                            xt[j][4 * q + l] = t[:, l * 512:(l + 1) * 512]
                    return emit

                def wq_dma_unit(q):
                    def emit():
                        nc.sync.dma_start(wqt[q][:], wq4[q])
                    return emit

                def mm_unit(g, d):
                    def emit():
                        if d == 0:
                            accs[g] = (accp.tile([128, 512], f32, tag="accA", name="accA"),
                                       accp.tile([128, 512], f32, tag="accB", name="accB"))
                        accA, accB = accs[g]
                        q, l = d // 4, d % 4
                        if g == 0:
                            wA = wkv[q][:, l * 256:l * 256 + 128]
                            wB = wkv[q][:, l * 256 + 128:l * 256 + 256]
                        else:
                            m = 2 * (g - 1)
                            wA = wqt[q][:, l * 512 + m * 128:l * 512 + (m + 1) * 128]
                            wB = wqt[q][:, l * 512 + (m + 1) * 128:l * 512 + (m + 2) * 128]
                        st = (d == 0)
                        sp_ = (d == DCH - 1)
                        nc.tensor.matmul(accA[:], wA, xt[j][d][:],
                                         start=st, stop=sp_)
                        nc.tensor.matmul(accB[:], wB, xt[j][d][:],
                                         start=st, stop=sp_)
                    return emit

                def vt_unit(g):
                    def emit():
                        accB = accs[g][1]
                        v_sb = stage.tile([128, 512], f32, tag="v_sb", bufs=2)
                        nc.scalar.copy(v_sb[:], accB[:])
                        vt_ps = tmpp.tile([128, 512], f32, tag="tmp")
                        for b in range(4):
                            nc.tensor.transpose(
                                vt_ps[:, b * 128:(b + 1) * 128],
                                v_sb[:, b * 128:(b + 1) * 128], ident[:])
                        nc.vector.tensor_copy(vnat[j][:], vt_ps[:])
                    return emit

                def rope_lazy(g, which, dst, eng):
                    def emit():
                        rope_unit(accs[g][which], dst, j, eng)()
                    return emit

                for q in range(8):
                    dma_units.append(dma_unit(q))
                if with_consts:
                    def cdma():
                        nc.sync.dma_start(cosT[:], cosT_d[:, :])
                        nc.sync.dma_start(sinT[:], sinT_d[:, :])
                        nc.sync.dma_start(rmat[:], rmat_d[:, :])
                        nc.sync.dma_start(ident[:], ident_d[:, :])
                        nc.sync.dma_start(tri[:], tri_d[:, :])
                        nc.sync.dma_start(ksT[:], ksT_d[:, :])
                        nc.sync.dma_start(ones128[:], ones128_d[:, :])
                    dma_units.append(cdma)
                    for q in range(8):
                        dma_units.append(wq_dma_unit(q))
                # group 0: (k, v); group 1: (q0, q1); group 2: (q2, q3)
                for d in range(DCH):
                    units.append(mm_unit(0, d))
                units.append(rope_lazy(0, 0, krot[j], 0))
                units.append(vt_unit(0))
                for d in range(DCH):
                    units.append(mm_unit(1, d))
                units.append(rope_lazy(1, 0, qrot[0][j], 1))
                units.append(rope_lazy(1, 1, qrot[1][j], 0))
                for d in range(DCH):
                    units.append(mm_unit(2, d))
                units.append(rope_lazy(2, 0, qrot[2][j], 1))
                units.append(rope_lazy(2, 1, qrot[3][j], 0))
                return dma_units, units

            def b_units(j):
                """Attention emission units for seq-tile j."""
                units = []
                ni = 4 * j + 4
                state = {}

                def av_mm(h, i):
                    delta = i - 4 * j
                    c0 = 128 * delta if delta > 0 else 0
                    nc.tensor.matmul(
                        state["av"][:, c0:512],
                        vnat[i // 4][:, (i % 4) * 128:(i % 4 + 1) * 128],
                        state["e"][i][:, c0:512],
                        start=(i == 0), stop=(i == ni - 1),
                        skip_group_check=True)

                def tile_unit(h, i):
                    def emit():
                        if i == 0:
                            state["av"] = avp.tile([128, 512], f32, tag="av", name="av")
                            state["acc_e"] = stage.tile([128, 512], bf16,
                                                        tag="acc_e", bufs=2, name="acc_e")
                            state["e"] = {}
                        acc_av = state["av"]
                        acc_e = state["acc_e"]
                        delta = i - 4 * j
                        c0 = 128 * delta if delta > 0 else 0
                        s_ps = sp.tile([128, 512], f32, tag="s")
                        nc.tensor.matmul(
                            s_ps[:, c0:512],
                            krot[i // 4][:, (i % 4) * 128:(i % 4 + 1) * 128],
                            qrot[h][j][:, c0:512], start=True, stop=True)
                        if i > 0:
                            av_mm(h, i - 1)
                        e_sb = stage.tile([128, 512], bf16, tag="e", bufs=6)
                        # region enhance/suppress folded into exp's scale
                        if i < 5 and j >= 2:
                            nc.scalar.activation(e_sb[:, c0:512],
                                                 s_ps[:, c0:512], EXP,
                                                 scale=ksT[:, i:i + 1])
                        elif i < 5 and j == 1:
                            cs = BOUND - 512   # 99: rows >= BOUND scaled
                            nc.scalar.activation(e_sb[:, c0:cs],
                                                 s_ps[:, c0:cs], EXP)
                            nc.scalar.activation(e_sb[:, cs:512],
                                                 s_ps[:, cs:512], EXP,
                                                 scale=ksT[:, i:i + 1])
                        else:
                            nc.scalar.activation(e_sb[:, c0:512],
                                                 s_ps[:, c0:512], EXP)
                        if delta >= 0:
                            # causal mask on the partial 128-col block
                            nc.vector.tensor_mul(e_sb[:, c0:c0 + 128],
                                                 e_sb[:, c0:c0 + 128], tri[:])
                        # denominator accumulation on DVE (bf16, 2x rate)
                        if i == 0:
                            nc.vector.tensor_copy(acc_e[:], e_sb[:])
                        else:
                            nc.vector.tensor_add(acc_e[:, c0:512],
                                                 acc_e[:, c0:512],
                                                 e_sb[:, c0:512])
                        state["e"][i] = e_sb
                    return emit

                def fin_unit(h):
                    def emit():
                        acc_av = state["av"]
                        acc_e = state["acc_e"]
                        av_mm(h, ni - 1)
                        # denom replicated to all partitions via ones matmul;
                        # 1/x = exp(-ln(x)) on ACT (DVE reciprocal is 3.4us)
                        dn_ps = tmpp.tile([128, 512], f32, tag="tmp")
                        nc.tensor.matmul(dn_ps[:], ones128[:], acc_e[:],
                                         start=True, stop=True)
                        lrec = stage.tile([128, 512], f32, tag="lrec", bufs=2)
                        nc.scalar.activation(lrec[:], dn_ps[:], LN)
                        rec = stage.tile([128, 512], f32, tag="rec", bufs=2)
                        nc.scalar.activation(rec[:], lrec[:], EXP, scale=-1.0)
                        nc.vector.tensor_mul(attn[h][j][:], acc_av[:], rec[:])
                    return emit

                for h in range(QH):
                    for i in range(ni):
                        units.append(tile_unit(h, i))
                    units.append(fin_unit(h))
                return units

            def c_units(n_range, t_range, par):
                """o_proj emission units; o_ps borrows the idle accp banks."""
                units = []
                wo_t = {}

                def wo_dma(n):
                    def emit():
                        t = wop.tile([128, 2048], bf16, tag="wo", bufs=2,
                                     name="wo")
                        nc.sync.dma_start(t[:], woT[n])
                        wo_t[n] = t
                    return emit

                o_quad = {}

                def ct_unit(n, t_, k, last=False):
                    def emit():
                        ti = t_ % 4
                        if ti == 0:
                            o_quad[0] = stage.tile([128, 2048], bf16,
                                                   tag="o_sb", bufs=2,
                                                   name="o_sb")
                        o_sb = o_quad[0]
                        tag = "accA" if k % 2 == 0 else "accB"
                        o_ps = accp.tile([128, 512], f32, tag=tag)
                        for h in range(QH):
                            nc.tensor.matmul(
                                o_ps[:],
                                attn[h][t_ // 4][:,
                                                 (t_ % 4) * 128:(t_ % 4 + 1) * 128],
                                wo_t[n][:, h * 512:(h + 1) * 512],
                                start=(h == 0), stop=(h == QH - 1))
                        dst = o_sb[:, ti * 512:(ti + 1) * 512]
                        if k % 2 == 0:
                            nc.scalar.copy(dst, o_ps[:])
                        else:
                            nc.vector.tensor_copy(dst, o_ps[:])
                        if last:
                            nc.sync.dma_start(
                                out[n, t_ // 4, :, ti * 512:(ti + 1) * 512],
                                dst)
                        elif ti == 3:
                            nc.sync.dma_start(out[n, t_ // 4], o_sb[:])
                    return emit

                k = par
                n_list = list(n_range)
                units.append(wo_dma(n_list[0]))
                for ii, n in enumerate(n_list):
                    if ii + 1 < len(n_list):
                        units.append(wo_dma(n_list[ii + 1]))
                    for t_ in t_range:
                        last = (ii == len(n_list) - 1 and
                                t_ == list(t_range)[-1] - 0 and
                                list(t_range)[-1] == t_ and len(t_range) == 4)
                        units.append(ct_unit(n, t_, k, last=(ii == len(n_list) - 1 and len(t_range) == 4)))
                        k += 1
                return units

            # ---- emission schedule: A(0), then B(j) || A(j+1), B(3) || C ----
            d0, m0 = a_units(0, with_consts=True)
            for u in d0 + m0:
                u()
            for j in range(J):
                main = b_units(j)
                if j + 1 < J:
                    dma_f, fill = a_units(j + 1)
                else:
                    dma_f, fill = [], c_units(range(8), range(12), 0)
                hold = min(6, len(main) - 1)
                for u in dma_f + main[:hold] + _interleave(main[hold:], fill):
                    u()
            # o_proj tail: t 12..15 for all n (wo re-streamed)
            for u in c_units(range(7, -1, -1), range(12, 16), 0):
                u()

    # Split multi-wait instructions onto standalone EventSemaphore
    # instructions.
    import bass_rust
    bass_rust.generate_event_semaphores(nc)
    return nc


def _get_compiled():
    if "nc" not in _CACHE:
        _CACHE["nc"] = _build_bass()
        _CACHE["const"] = _host_constants()
    return _CACHE["nc"], _CACHE["const"]


def kernel(hidden_states, wq, wk, wv, wo, _trace=False):
    from concourse.bass_utils import run_bass_kernel_spmd

    nc, cst = _get_compiled()

    x = np.asarray(hidden_states, dtype=np.float32).reshape(S, D)
    xT = np.ascontiguousarray(x.T)                       # [D, S]
    # [j, dquad, p, (d0 f | d1 f | d2 f | d3 f)] -- 4KB lines, one DMA each
    xTt = xT.reshape(8, 4, 128, J, 512).transpose(3, 0, 2, 1, 4)
    xTt = _bf16(xTt.reshape(J, 8, 128, 2048))
    wq = np.asarray(wq, dtype=np.float32)
    wk = np.asarray(wk, dtype=np.float32)
    wv = np.asarray(wv, dtype=np.float32)
    wo = np.asarray(wo, dtype=np.float32)
    scale = 1.0 / math.sqrt(HD)

    in_maps = []
    for d in range(NCORES):
        wq_d = (wq[d * QH * HD:(d + 1) * QH * HD] * scale).T  # [D, 512]
        wk_d = wk[d * HD:(d + 1) * HD].T                      # [D, 128]
        wv_d = wv[d * HD:(d + 1) * HD].T                      # [D, 128]
        wkv_d = np.concatenate([wk_d, wv_d], axis=1)       # [D, 256]
        wkv4_d = wkv_d.reshape(8, 4, 128, 256).transpose(0, 2, 1, 3).reshape(
            8, 128, 1024)
        wq4_d = wq_d.reshape(8, 4, 128, 512).transpose(0, 2, 1, 3).reshape(
            8, 128, 2048)
        wo_d = wo[:, d * QH * HD:(d + 1) * QH * HD].T         # [512, D]
        # [n, p, (h0 f | h1 f | h2 f | h3 f)] -- one DMA per n
        woT_d = np.ascontiguousarray(
            wo_d.reshape(QH, 128, 8, 512).transpose(2, 1, 0, 3).reshape(
                8, 128, 2048))
        in_maps.append({
            "xTt": xTt,
            "wkv4": _bf16(wkv4_d),
            "wq4": _bf16(wq4_d),
            "woT": _bf16(woT_d),
            "cosT": cst["cosT"], "sinT": cst["sinT"],
            "rmat": cst["rmat"], "ident": cst["ident"],
            "tri": cst["tri"], "ksT": cst["ksT"],
            "ones128": cst["ones128"],
        })

    res = run_bass_kernel_spmd(nc, in_maps, core_ids=list(range(NCORES)),
                               trace=_trace)
    acc = res.results[0]["out"].astype(np.float64)
    for d in range(1, NCORES):
        acc += res.results[d]["out"].astype(np.float64)
    # out[n, tq, p, ti*512+f] -> out[128*(4tq+ti)+p, 512n+f]
    acc = acc.reshape(8, 4, 128, 4, 512)        # [n, tq, p, ti, f]
    outp = acc.transpose(1, 3, 2, 0, 4).reshape(S, D).astype(np.float32)
    outp = outp.reshape(1, S, D)
    if _trace:
        _CACHE["last_results"] = res
    return outp


# revision 11
# speedup vs baseline: 1.0099x; 1.0099x over previous
"""Trainium2 Bass kernel for nn_AttnAdapter: GQA attention with RoPE,
region-based enhance/suppress score scaling, causal mask, o_proj.

Sharding: tensor-parallel over heads across 8 NeuronCores. Core d holds
q-heads 4d..4d+3 (wq rows), kv-head d (wk/wv rows), and wo columns
512d..512(d+1). Each core computes a full [S, D] partial of the output;
the host sums the 8 partials (the TP all-reduce, done at unshard time).

v3: all-bf16 matmuls (FWL weight loads), weights loaded once, software-
pipelined emission: projection matmuls for seq-tile j+1 are interleaved
into the attention stream for seq-tile j (and the first o_proj tiles
into the last attention tile) so the PE never stalls on the exp stream.
Softmax denom is accumulated on DVE in bf16, replicated across
partitions with a ones-matmul, and inverted with ACT Ln/Exp (the DVE
reciprocal costs 3.4us/tile). Outputs are stored bf16 and summed on
host.
"""

import math

import numpy as np

# ---- problem constants (hardcoded; kernel.py must be self-contained) ----
S = 2048          # sequence length
D = 4096          # model dim
HD = 128          # head dim
NCORES = 8
QH = 4            # q heads per core
SYS_LEN, IMG_LEN = 35, 576
BOUND = SYS_LEN + IMG_LEN          # 611
ENH, SUP = 1.5, 0.5
ROPE_BASE = 10000.0

J = 4             # sq tiles of 512
NSK = 16          # sk tiles of 128
DCH = 32          # D chunks of 128

_CACHE = {}


def _bf16(x):
    import ml_dtypes
    return np.ascontiguousarray(np.asarray(x, dtype=ml_dtypes.bfloat16))


def _host_constants():
    inv_freq = 1.0 / (ROPE_BASE ** (np.arange(0, HD, 2, dtype=np.float32) / HD))
    pos = np.arange(S, dtype=np.float32)
    freqs = pos[:, None] * inv_freq[None, :]              # [S, 64]
    emb = np.concatenate([freqs, freqs], axis=-1)         # [S, 128]
    cosT = _bf16(np.cos(emb).T)                           # [128, S]
    sinT = _bf16(np.sin(emb).T)

    # rotate_half as a matmul: rot = R @ q (in [hd, s] layout).
    # matmul(out, lhsT, rhs) = lhsT.T @ rhs, so feed RT = R.T.
    RT = np.zeros((HD, HD), dtype=np.float32)
    half = HD // 2
    for c in range(half):
        RT[c + half, c] = -1.0      # rot[c] = -q[c+64]
    for c in range(half, HD):
        RT[c - half, c] = 1.0       # rot[c] = q[c-64]
    rmat = _bf16(RT)

    ident = np.eye(HD, dtype=np.float32)

    # Diagonal-tile causal mask [128, 128]: within the first 128-col block
    # of a diagonal tile, col c valid iff c >= p. Same for every delta.
    p = np.arange(128)[:, None]
    c = np.arange(128)[None, :]
    tri = _bf16((c >= p).astype(np.float32))

    # key_scale in partition layout per sk-tile: ksT[p, i] = scale(128*i+p)
    kpos = np.arange(S)
    key_scale = np.where(kpos < SYS_LEN, SUP,
                         np.where(kpos < BOUND, ENH, 1.0)).astype(np.float32)
    ksT = np.ascontiguousarray(key_scale[:5 * 128].reshape(5, 128).T)  # [128, 5]

    ones128 = _bf16(np.ones((HD, HD), dtype=np.float32))
    return dict(cosT=cosT, sinT=sinT, rmat=rmat, ident=ident, tri=tri,
                ksT=ksT, ones128=ones128)


def _interleave(main, fill):
    """Merge two unit lists, spreading `fill` evenly across `main`."""
    units = []
    nf = len(fill)
    nm = max(1, len(main))
    k = 0
    for m, u in enumerate(main):
        units.append(u)
        want = (m + 1) * nf // nm
        while k < want:
            units.append(fill[k])
            k += 1
    units.extend(fill[k:])
    return units


def _build_bass():
    import concourse.bass as bass
    import concourse.mybir as mybir
    from concourse.tile import TileContext
    from contextlib import ExitStack

    f32 = mybir.dt.float32
    bf16 = mybir.dt.bfloat16

    nc = bass.Bass()
    # xTt[j, d, p, f] = x.T[128d+p, 512j+f] -- each (j,d) tile contiguous
    xTt = nc.dram_tensor("xTt", [J, 8, 128, 2048], bf16, kind="ExternalInput")
    # wkv4[q, p, l*256+c]: c 0:128=wkT chunk of d=4q+l, 128:256=wvT chunk
    wkv4 = nc.dram_tensor("wkv4", [8, 128, 1024], bf16, kind="ExternalInput")
    # wq4[q, p, l*512+c]: wqT chunk (4 heads x 128) of d=4q+l
    wq4 = nc.dram_tensor("wq4", [8, 128, 2048], bf16, kind="ExternalInput")
    # woT[n, h, p, f] = woT[128h+p, 512n+f]
    woT = nc.dram_tensor("woT", [8, 128, 2048], bf16, kind="ExternalInput")
    cosT_d = nc.dram_tensor("cosT", [HD, S], bf16, kind="ExternalInput")
    sinT_d = nc.dram_tensor("sinT", [HD, S], bf16, kind="ExternalInput")
    rmat_d = nc.dram_tensor("rmat", [HD, HD], bf16, kind="ExternalInput")
    ident_d = nc.dram_tensor("ident", [HD, HD], f32, kind="ExternalInput")
    tri_d = nc.dram_tensor("tri", [HD, HD], bf16, kind="ExternalInput")
    ksT_d = nc.dram_tensor("ksT", [HD, 5], f32, kind="ExternalInput")
    ones128_d = nc.dram_tensor("ones128", [HD, HD], bf16, kind="ExternalInput")
    # out[n, t, p, f] = out[128t+p, 512n+f], bf16 partial (host sums cores)
    out = nc.dram_tensor("out", [8, 4, 128, 2048], bf16, kind="ExternalOutput")

    EXP = mybir.ActivationFunctionType.Exp
    LN = mybir.ActivationFunctionType.Ln

    with TileContext(nc) as tc, ExitStack() as ctx:
        const = ctx.enter_context(tc.tile_pool(name="const", bufs=1))
        wpool = ctx.enter_context(tc.tile_pool(name="wpool", bufs=1))
        persist = ctx.enter_context(tc.tile_pool(name="persist", bufs=1))

        wkv = [wpool.tile([128, 1024], bf16, name=f"wkv{q}") for q in range(8)]
        wqt = [wpool.tile([128, 2048], bf16, name=f"wqt{q}") for q in range(8)]
        qrot = [[persist.tile([HD, 512], bf16, name=f"qrot{m}_{j}")
                 for j in range(J)] for m in range(QH)]
        krot = [persist.tile([HD, 512], bf16, name=f"krot{j}") for j in range(J)]
        vnat = [persist.tile([HD, 512], bf16, name=f"vnat{j}") for j in range(J)]
        attn = [[persist.tile([HD, 512], bf16, name=f"attn{h}_{j}")
                 for j in range(J)] for h in range(QH)]

        cosT = const.tile([HD, S], bf16)
        sinT = const.tile([HD, S], bf16)
        rmat = const.tile([HD, HD], bf16)
        ident = const.tile([HD, HD], f32)
        tri = const.tile([HD, HD], bf16)
        ksT = const.tile([HD, 5], f32)
        ones128 = const.tile([HD, HD], bf16)

        with tc.tile_pool(name="xp", bufs=2) as xp, \
             tc.tile_pool(name="wop", bufs=1) as wop, \
             tc.tile_pool(name="accp", bufs=1, space="PSUM") as accp, \
             tc.tile_pool(name="tmpp", bufs=1, space="PSUM") as tmpp, \
             tc.tile_pool(name="sp", bufs=3, space="PSUM") as sp, \
             tc.tile_pool(name="avp", bufs=2, space="PSUM") as avp, \
             tc.tile_pool(name="stage", bufs=1) as stage:

            xt = [[None] * DCH for _ in range(J)]

            def rope_unit(acc, dst, j, eng):
                def emit():
                    sq = slice(j * 512, (j + 1) * 512)
                    q_sb = stage.tile([128, 512], bf16, tag="q_sb", bufs=3)
                    # release copy frees the acc bank; alternate queues so it
                    # is not stuck behind the exp stream
                    if eng == 0:
                        nc.scalar.copy(q_sb[:], acc[:])
                    else:
                        nc.vector.tensor_copy(q_sb[:], acc[:])
                    rot_ps = tmpp.tile([128, 512], f32, tag="tmp")
                    nc.tensor.matmul(rot_ps[:], rmat[:], q_sb[:],
                                     start=True, stop=True)
                    t2 = stage.tile([128, 512], f32, tag="t2", bufs=2)
                    nc.vector.tensor_mul(t2[:], rot_ps[:], sinT[:, sq])
                    nc.vector.tensor_mul(dst[:], q_sb[:], cosT[:, sq])
                    nc.vector.tensor_add(dst[:], dst[:], t2[:])
                return emit

            def a_units(j, with_consts=False):
                """Projection+RoPE units for seq-tile j: (dma_units, main)."""
                dma_units = []
                units = []
                accs = {}

                def dma_unit(q):
                    def emit():
                        t = xp.tile([128, 2048], bf16, tag=f"x{q}",
                                    bufs=2, name=f"x{q}")
                        if j == 0 and q == 0:
                            # fine-grained first chunks: first matmul can
                            # start after ~192KB instead of ~768KB
                            for l in range(4):
                                cs = slice(l * 256, (l + 1) * 256)
                                nc.sync.dma_start(wkv[q][:, cs], wkv4[q, :, cs])
                                xs = slice(l * 512, (l + 1) * 512)
                                nc.sync.dma_start(t[:, xs], xTt[j, q, :, xs])
                        else:
                            if j == 0:
                                nc.sync.dma_start(wkv[q][:], wkv4[q])
                            nc.sync.dma_start(t[:], xTt[j, q])
                        for l in range(4):
                            xt[j][4 * q + l] = t[:, l * 512:(l + 1) * 512]
                    return emit

                def wq_dma_unit(q):
                    def emit():
                        nc.sync.dma_start(wqt[q][:], wq4[q])
                    return emit

                def wslice(g, q, l):
                    if g == 0:
                        return (wkv[q][:, l * 256:l * 256 + 128],
                                wkv[q][:, l * 256 + 128:l * 256 + 256])
                    m = 2 * (g - 1)
                    return (wqt[q][:, l * 512 + m * 128:l * 512 + (m + 1) * 128],
                            wqt[q][:, l * 512 + (m + 1) * 128:l * 512 + (m + 2) * 128])

                def mm6_unit(d):
                    def emit():
                        if d == 0:
                            accs[0] = (accp.tile([128, 512], f32, tag="accA", name="accA"),
                                       accp.tile([128, 512], f32, tag="accB", name="accB"))
                            accs[1] = (avp.tile([128, 512], f32, tag="av", name="av"),
                                       avp.tile([128, 512], f32, tag="av", name="av"))
                            accs[2] = (sp.tile([128, 512], f32, tag="s", name="s"),
                                       sp.tile([128, 512], f32, tag="s", name="s"))
                        q, l = d // 4, d % 4
                        st = (d == 0)
                        sp_ = (d == DCH - 1)
                        for g in range(3):
                            wA, wB = wslice(g, q, l)
                            nc.tensor.matmul(accs[g][0][:], wA, xt[j][d][:],
                                             start=st, stop=sp_)
                            nc.tensor.matmul(accs[g][1][:], wB, xt[j][d][:],
                                             start=st, stop=sp_)
                    return emit

                def mm_unit(g, d):
                    def emit():
                        if d == 0:
                            accs[g] = (accp.tile([128, 512], f32, tag="accA", name="accA"),
                                       accp.tile([128, 512], f32, tag="accB", name="accB"))
                        accA, accB = accs[g]
                        q, l = d // 4, d % 4
                        wA, wB = wslice(g, q, l)
                        st = (d == 0)
                        sp_ = (d == DCH - 1)
                        nc.tensor.matmul(accA[:], wA, xt[j][d][:],
                                         start=st, stop=sp_)
                        nc.tensor.matmul(accB[:], wB, xt[j][d][:],
                                         start=st, stop=sp_)
                    return emit

                def vt_unit(g):
                    def emit():
                        accB = accs[g][1]
                        v_sb = stage.tile([128, 512], f32, tag="v_sb", bufs=2)
                        nc.scalar.copy(v_sb[:], accB[:])
                        vt_ps = tmpp.tile([128, 512], f32, tag="tmp")
                        for b in range(4):
                            nc.tensor.transpose(
                                vt_ps[:, b * 128:(b + 1) * 128],
                                v_sb[:, b * 128:(b + 1) * 128], ident[:])
                        nc.vector.tensor_copy(vnat[j][:], vt_ps[:])
                    return emit

                def rope_lazy(g, which, dst, eng):
                    def emit():
                        rope_unit(accs[g][which], dst, j, eng)()
                    return emit

                for q in range(8):
                    dma_units.append(dma_unit(q))
                if with_consts:
                    def cdma():
                        nc.sync.dma_start(cosT[:], cosT_d[:, :])
                        nc.sync.dma_start(sinT[:], sinT_d[:, :])
                        nc.sync.dma_start(rmat[:], rmat_d[:, :])
                        nc.sync.dma_start(ident[:], ident_d[:, :])
                        nc.sync.dma_start(tri[:], tri_d[:, :])
                        nc.sync.dma_start(ksT[:], ksT_d[:, :])
                        nc.sync.dma_start(ones128[:], ones128_d[:, :])
                    dma_units.append(cdma)
                    for q in range(8):
                        dma_units.append(wq_dma_unit(q))
                # group 0: (k, v); group 1: (q0, q1); group 2: (q2, q3)
                if j == 0:
                    # startup is DMA-paced and attention has not begun:
                    # borrow the av/s banks and do all 6 outputs in one pass
                    for d in range(DCH):
                        units.append(mm6_unit(d))
                    units.append(rope_lazy(0, 0, krot[j], 0))
                    units.append(vt_unit(0))
                    units.append(rope_lazy(1, 0, qrot[0][j], 1))
                    units.append(rope_lazy(1, 1, qrot[1][j], 0))
                    units.append(rope_lazy(2, 0, qrot[2][j], 1))
                    units.append(rope_lazy(2, 1, qrot[3][j], 0))
                    return dma_units, units
                for d in range(DCH):
                    units.append(mm_unit(0, d))
                units.append(rope_lazy(0, 0, krot[j], 0))
                units.append(vt_unit(0))
                for d in range(DCH):
                    units.append(mm_unit(1, d))
                units.append(rope_lazy(1, 0, qrot[0][j], 1))
                units.append(rope_lazy(1, 1, qrot[1][j], 0))
                for d in range(DCH):
                    units.append(mm_unit(2, d))
                units.append(rope_lazy(2, 0, qrot[2][j], 1))
                units.append(rope_lazy(2, 1, qrot[3][j], 0))
                return dma_units, units

            def b_units(j):
                """Attention emission units for seq-tile j."""
                units = []
                ni = 4 * j + 4
                state = {}

                def av_mm(h, i):
                    delta = i - 4 * j
                    c0 = 128 * delta if delta > 0 else 0
                    nc.tensor.matmul(
                        state["av"][:, c0:512],
                        vnat[i // 4][:, (i % 4) * 128:(i % 4 + 1) * 128],
                        state["e"][i][:, c0:512],
                        start=(i == 0), stop=(i == ni - 1),
                        skip_group_check=True)

                def tile_unit(h, i):
                    def emit():
                        if i == 0:
                            state["av"] = avp.tile([128, 512], f32, tag="av", name="av")
                            state["acc_e"] = stage.tile([128, 512], bf16,
                                                        tag="acc_e", bufs=2, name="acc_e")
                            state["e"] = {}
                        acc_av = state["av"]
                        acc_e = state["acc_e"]
                        delta = i - 4 * j
                        c0 = 128 * delta if delta > 0 else 0
                        s_ps = sp.tile([128, 512], f32, tag="s")
                        nc.tensor.matmul(
                            s_ps[:, c0:512],
                            krot[i // 4][:, (i % 4) * 128:(i % 4 + 1) * 128],
                            qrot[h][j][:, c0:512], start=True, stop=True)
                        if i > 0:
                            av_mm(h, i - 1)
                        e_sb = stage.tile([128, 512], bf16, tag="e", bufs=6)
                        # region enhance/suppress folded into exp's scale
                        if i < 5 and j >= 2:
                            nc.scalar.activation(e_sb[:, c0:512],
                                                 s_ps[:, c0:512], EXP,
                                                 scale=ksT[:, i:i + 1])
                        elif i < 5 and j == 1:
                            cs = BOUND - 512   # 99: rows >= BOUND scaled
                            nc.scalar.activation(e_sb[:, c0:cs],
                                                 s_ps[:, c0:cs], EXP)
                            nc.scalar.activation(e_sb[:, cs:512],
                                                 s_ps[:, cs:512], EXP,
                                                 scale=ksT[:, i:i + 1])
                        else:
                            nc.scalar.activation(e_sb[:, c0:512],
                                                 s_ps[:, c0:512], EXP)
                        if delta >= 0:
                            # causal mask on the partial 128-col block
                            nc.vector.tensor_mul(e_sb[:, c0:c0 + 128],
                                                 e_sb[:, c0:c0 + 128], tri[:])
                        # denominator accumulation on DVE (bf16, 2x rate)
                        if i == 0:
                            nc.vector.tensor_copy(acc_e[:], e_sb[:])
                        else:
                            nc.vector.tensor_add(acc_e[:, c0:512],
                                                 acc_e[:, c0:512],
                                                 e_sb[:, c0:512])
                        state["e"][i] = e_sb
                    return emit

                def fin_unit(h):
                    def emit():
                        acc_av = state["av"]
                        acc_e = state["acc_e"]
                        av_mm(h, ni - 1)
                        # denom replicated to all partitions via ones matmul;
                        # 1/x = exp(-ln(x)) on ACT (DVE reciprocal is 3.4us)
                        dn_ps = tmpp.tile([128, 512], f32, tag="tmp")
                        nc.tensor.matmul(dn_ps[:], ones128[:], acc_e[:],
                                         start=True, stop=True)
                        lrec = stage.tile([128, 512], f32, tag="lrec", bufs=2)
                        nc.scalar.activation(lrec[:], dn_ps[:], LN)
                        rec = stage.tile([128, 512], f32, tag="rec", bufs=2)
                        nc.scalar.activation(rec[:], lrec[:], EXP, scale=-1.0)
                        nc.vector.tensor_mul(attn[h][j][:], acc_av[:], rec[:])
                    return emit

                for h in range(QH):
                    for i in range(ni):
                        units.append(tile_unit(h, i))
                    units.append(fin_unit(h))
                return units

            def c_units(n_range, t_range, par):
                """o_proj emission units; o_ps borrows the idle accp banks."""
                units = []
                wo_t = {}

                def wo_dma(n):
                    def emit():
                        t = wop.tile([128, 2048], bf16, tag="wo", bufs=2,
                                     name="wo")
                        nc.sync.dma_start(t[:], woT[n])
                        wo_t[n] = t
                    return emit

                o_quad = {}

                def ct_unit(n, t_, k, last=False):
                    def emit():
                        ti = t_ % 4
                        if ti == 0:
                            o_quad[0] = stage.tile([128, 2048], bf16,
                                                   tag="o_sb", bufs=2,
                                                   name="o_sb")
                        o_sb = o_quad[0]
                        tag = "accA" if k % 2 == 0 else "accB"
                        o_ps = accp.tile([128, 512], f32, tag=tag)
                        for h in range(QH):
                            nc.tensor.matmul(
                                o_ps[:],
                                attn[h][t_ // 4][:,
                                                 (t_ % 4) * 128:(t_ % 4 + 1) * 128],
                                wo_t[n][:, h * 512:(h + 1) * 512],
                                start=(h == 0), stop=(h == QH - 1))
                        dst = o_sb[:, ti * 512:(ti + 1) * 512]
                        if k % 2 == 0:
                            nc.scalar.copy(dst, o_ps[:])
                        else:
                            nc.vector.tensor_copy(dst, o_ps[:])
                        if last:
                            nc.sync.dma_start(
                                out[n, t_ // 4, :, ti * 512:(ti + 1) * 512],
                                dst)
                        elif ti == 3:
                            nc.sync.dma_start(out[n, t_ // 4], o_sb[:])
                    return emit

                k = par
                n_list = list(n_range)
                units.append(wo_dma(n_list[0]))
                for ii, n in enumerate(n_list):
                    if ii + 1 < len(n_list):
                        units.append(wo_dma(n_list[ii + 1]))
                    for t_ in t_range:
                        units.append(ct_unit(n, t_, k, last=(
                            ii == len(n_list) - 1 and len(t_range) == 4)))
                        k += 1
                return units

            # ---- emission schedule: A(0), then B(j) || A(j+1), B(3) || C ----
            d0, m0 = a_units(0, with_consts=True)
            for u in d0 + m0:
                u()
            for j in range(J):
                main = b_units(j)
                if j + 1 < J:
                    dma_f, fill = a_units(j + 1)
                else:
                    dma_f, fill = [], c_units(range(8), range(12), 0)
                hold = min(6, len(main) - 1)
                for u in dma_f + main[:hold] + _interleave(main[hold:], fill):
                    u()
            # o_proj tail: t 12..15 for all n (wo re-streamed)
            for u in c_units(range(7, -1, -1), range(12, 16), 0):
                u()

    # Split multi-wait instructions onto standalone EventSemaphore
    # instructions.
    import bass_rust
    bass_rust.generate_event_semaphores(nc)
    return nc


def _get_compiled():
    if "nc" not in _CACHE:
        _CACHE["nc"] = _build_bass()
        _CACHE["const"] = _host_constants()
    return _CACHE["nc"], _CACHE["const"]


def kernel(hidden_states, wq, wk, wv, wo, _trace=False):
    from concourse.bass_utils import run_bass_kernel_spmd

    nc, cst = _get_compiled()

    x = np.asarray(hidden_states, dtype=np.float32).reshape(S, D)
    xT = np.ascontiguousarray(x.T)                       # [D, S]
    # [j, dquad, p, (d0 f | d1 f | d2 f | d3 f)] -- 4KB lines, one DMA each
    xTt = xT.reshape(8, 4, 128, J, 512).transpose(3, 0, 2, 1, 4)
    xTt = _bf16(xTt.reshape(J, 8, 128, 2048))
    wq = np.asarray(wq, dtype=np.float32)
    wk = np.asarray(wk, dtype=np.float32)
    wv = np.asarray(wv, dtype=np.float32)
    wo = np.asarray(wo, dtype=np.float32)
    scale = 1.0 / math.sqrt(HD)

    in_maps = []
    for d in range(NCORES):
        wq_d = (wq[d * QH * HD:(d + 1) * QH * HD] * scale).T  # [D, 512]
        wk_d = wk[d * HD:(d + 1) * HD].T                      # [D, 128]
        wv_d = wv[d * HD:(d + 1) * HD].T                      # [D, 128]
        wkv_d = np.concatenate([wk_d, wv_d], axis=1)       # [D, 256]
        wkv4_d = wkv_d.reshape(8, 4, 128, 256).transpose(0, 2, 1, 3).reshape(
            8, 128, 1024)
        wq4_d = wq_d.reshape(8, 4, 128, 512).transpose(0, 2, 1, 3).reshape(
            8, 128, 2048)
        wo_d = wo[:, d * QH * HD:(d + 1) * QH * HD].T         # [512, D]
        # [n, p, (h0 f | h1 f | h2 f | h3 f)] -- one DMA per n
        woT_d = np.ascontiguousarray(
            wo_d.reshape(QH, 128, 8, 512).transpose(2, 1, 0, 3).reshape(
                8, 128, 2048))
        in_maps.append({
            "xTt": xTt,
            "wkv4": _bf16(wkv4_d),
            "wq4": _bf16(wq4_d),
            "woT": _bf16(woT_d),
            "cosT": cst["cosT"], "sinT": cst["sinT"],
            "rmat": cst["rmat"], "ident": cst["ident"],
            "tri": cst["tri"], "ksT": cst["ksT"],
            "ones128": cst["ones128"],
        })

    res = run_bass_kernel_spmd(nc, in_maps, core_ids=list(range(NCORES)),
                               trace=_trace)
    acc = res.results[0]["out"].astype(np.float64)
    for d in range(1, NCORES):
        acc += res.results[d]["out"].astype(np.float64)
    # out[n, tq, p, ti*512+f] -> out[128*(4tq+ti)+p, 512n+f]
    acc = acc.reshape(8, 4, 128, 4, 512)        # [n, tq, p, ti, f]
    outp = acc.transpose(1, 3, 2, 0, 4).reshape(S, D).astype(np.float32)
    outp = outp.reshape(1, S, D)
    if _trace:
        _CACHE["last_results"] = res
    return outp
